# Initial kernel scaffold
#
"""Bitpacked Bernoulli sampling kernel for Trainium2 (8 NeuronCores, SPMD).

Problem: probs [1, 32768] f32, count=256 -> output [32768, 256] uint8, where
byte (n, c) packs 8 Bernoulli(probs[n]) bits MSB-first.

Strategy (per the sharding hint): embarrassingly parallel over the probs axis.
Each of the 8 cores handles 4096 rows and draws its own counter-based RNG
stream: the GPSIMD (Pool) engine's hardware xorwow generator, seeded with
per-core, per-lane states derived deterministically from the problem seed.
On-device pipeline per tile of 512 rows:
    random [128, 8192] u32  ->  is_lt threshold (p * 2^32, f32)  -> uint8 bits
    -> 3-level strided shift/or pack -> [128, 1024] bytes -> DMA out.

The output is bit-reproducible: the HW xorwow stream matches the rocRAND
recurrence (verified on-device), so `model_output()` below predicts the
kernel's output exactly in numpy.
"""
import sys

for _p in ("/opt/trn_rl_repo", "/root/.axon_site/_ro/trn_rl_repo"):
    if _p not in sys.path:
        sys.path.insert(0, _p)

import numpy as np

N_ROWS = 32768
COUNT = 256
N_CORES = 8
ROWS_PER_CORE = N_ROWS // N_CORES          # 4096
N_TILES = 8                                # tiles per core
ROWS_PER_TILE = ROWS_PER_CORE // N_TILES   # 512 rows: 128 partitions x 4 segments
SEGS = 4                                   # rows per partition per tile
BITS_PER_ROW = COUNT * 8                   # 2048
TILE_F = SEGS * BITS_PER_ROW               # 8192 uniforms per partition per tile
SEED = 1234

_CACHE = {}


def _build_program():
    import concourse.bass as bass
    import concourse.mybir as mybir
    import concourse.tile as tile

    Alu = mybir.AluOpType
    u32 = mybir.dt.uint32
    u8 = mybir.dt.uint8
    f32 = mybir.dt.float32

    def stt_imm(eng, out, in0, v, in1, op0, op1, imm_dtype):
        return eng.add_instruction(
            mybir.InstTensorScalarPtr(
                name=eng.bass.get_next_instruction_name(),
                is_scalar_tensor_tensor=True,
                op0=op0, op1=op1,
                ins=[eng.lower_ap(in0),
                     mybir.ImmediateValue(dtype=imm_dtype, value=v),
                     eng.lower_ap(in1)],
                outs=[eng.lower_ap(out)],
            )
        )

    nc = bass.Bass("TRN2", target_bir_lowering=False, debug=False)
    thr_d = nc.dram_tensor("thr", [128, N_TILES * SEGS], f32, kind="ExternalInput")
    st_d = nc.dram_tensor("st", [128, 6], u32, kind="ExternalOutput" if False else "ExternalInput")
    out_d = nc.dram_tensor("out", [ROWS_PER_CORE, COUNT], u8, kind="ExternalOutput")

    with tile.TileContext(nc) as tc:
        with (
            tc.tile_pool(name="io", bufs=1) as iop,
            tc.tile_pool(name="rnd", bufs=2) as rp,
            tc.tile_pool(name="bits", bufs=2) as bp,
        ):
            thr_t = iop.tile([128, N_TILES * SEGS], f32, name="thr_t")
            st_t = iop.tile([128, 6], u32, name="st_t")
            nc.sync.dma_start(thr_t[:], thr_d[:, :])
            nc.sync.dma_start(st_t[:], st_d[:, :])

            prev = nc.gpsimd.set_rand_state(st_t[:])
            for t in range(N_TILES):
                r = rp.tile([128, TILE_F], u32, name="r", tag="r")
                cur = nc.gpsimd.random(r[:])
                # RNG state is not a tile: order the stream explicitly.
                bass._add_dep_helper(cur.ins, prev.ins, sync=True,
                                     reason="xorwow stream order")
                prev = cur

                bits = bp.tile([128, TILE_F], u8, name="bits", tag="bits")
                for s in range(SEGS):
                    nc.vector.tensor_scalar(
                        bits[:, s * BITS_PER_ROW:(s + 1) * BITS_PER_ROW],
                        r[:, s * BITS_PER_ROW:(s + 1) * BITS_PER_ROW],
                        thr_t[:, t * SEGS + s: t * SEGS + s + 1],
                        None, Alu.is_lt,
                    )
                p1 = bp.tile([128, TILE_F // 2], u8, name="p1", tag="p1")
                stt_imm(nc.vector, p1[:], bits[:, 0:TILE_F:2], 1,
                        bits[:, 1:TILE_F:2], Alu.logical_shift_left, Alu.bitwise_or, u8)
                p2 = bp.tile([128, TILE_F // 4], u8, name="p2", tag="p2")
                stt_imm(nc.vector, p2[:], p1[:, 0:TILE_F // 2:2], 2,
                        p1[:, 1:TILE_F // 2:2], Alu.logical_shift_left, Alu.bitwise_or, u8)
                pk = bp.tile([128, TILE_F // 8], u8, name="pk", tag="pk")
                stt_imm(nc.vector, pk[:], p2[:, 0:TILE_F // 4:2], 4,
                        p2[:, 1:TILE_F // 4:2], Alu.logical_shift_left, Alu.bitwise_or, u8)

                # rows t*512 + p*4 + s are contiguous blocks of 1024 bytes per p
                dst = bass.AP(out_d, t * ROWS_PER_TILE * COUNT,
                              [[SEGS * COUNT, 128], [1, SEGS * COUNT]])
                nc.sync.dma_start(dst, pk[:])

    return nc


def make_states(seed=SEED):
    """Per-core, per-lane xorwow states [N_CORES, 128, 6] uint32, deterministic."""
    ss = np.random.SeedSequence(seed)
    gen = np.random.Generator(np.random.PCG64(ss))
    st = gen.integers(1, 2**32 - 1, size=(N_CORES, 128, 6), dtype=np.uint32)
    return st


def make_thresholds(probs):
    """Thresholds p * 2^32 (f32) laid out [core][128, 32] matching the kernel."""
    # row = core*4096 + t*512 + p*4 + s ; thr[core][p, t*4+s]
    v = (probs.reshape(N_CORES, N_TILES, 128, SEGS).astype(np.float64)
         * 4294967296.0).astype(np.float32)
    return np.ascontiguousarray(v.transpose(0, 2, 1, 3).reshape(N_CORES, 128, N_TILES * SEGS))


def xorwow_stream(state_p, n):
    """rocRAND xorwow outputs for a vector of lane states [L, 6] -> [L, n]."""
    x = [state_p[:, i].astype(np.uint32).copy() for i in range(5)]
    ctr = state_p[:, 5].astype(np.uint32).copy()
    out = np.empty((state_p.shape[0], n), dtype=np.uint32)
    with np.errstate(over='ignore'):
        for i in range(n):
            t = x[0] ^ (x[0] >> np.uint32(2))
            x[0], x[1], x[2], x[3] = x[1], x[2], x[3], x[4]
            x[4] = (x[4] ^ (x[4] << np.uint32(4))) ^ (t ^ (t << np.uint32(1)))
            ctr = ctr + np.uint32(362437)
            out[:, i] = x[4] + ctr
    return out


def model_output(probs, states=None):
    """Exact numpy prediction of the kernel's output (for validation)."""
    if states is None:
        states = make_states()
    thr = make_thresholds(probs)
    out = np.empty((N_ROWS, COUNT), dtype=np.uint8)
    w = (np.uint8(128) >> np.arange(8, dtype=np.uint8)).astype(np.uint8)
    for c in range(N_CORES):
        stream = xorwow_stream(states[c], N_TILES * TILE_F)   # [128, 65536]
        for t in range(N_TILES):
            r = stream[:, t * TILE_F:(t + 1) * TILE_F]        # [128, 8192]
            thr_cols = thr[c][:, t * SEGS:(t + 1) * SEGS]     # [128, 4]
            rf = r.astype(np.float32).reshape(128, SEGS, BITS_PER_ROW)
            bits = (rf < thr_cols[:, :, None]).astype(np.uint8)
            byts = (bits.reshape(128, SEGS, COUNT, 8) * w).sum(axis=-1).astype(np.uint8)
            base = c * ROWS_PER_CORE + t * ROWS_PER_TILE
            # row = base + p*4 + s
            out[base:base + ROWS_PER_TILE] = byts.reshape(128 * SEGS, COUNT)
    return out


def _get_program():
    if "nc" not in _CACHE:
        _CACHE["nc"] = _build_program()
    return _CACHE["nc"]


def kernel(probs_batch, count):
    probs_batch = np.asarray(probs_batch)
    assert probs_batch.shape == (1, N_ROWS), probs_batch.shape
    assert int(count) == COUNT, count
    probs = probs_batch.reshape(N_ROWS).astype(np.float32)

    from concourse.bass_utils import run_bass_kernel_spmd

    nc = _get_program()
    thr = make_thresholds(probs)
    states = make_states()
    in_maps = [
        {"thr": np.ascontiguousarray(thr[c]),
         "st": np.ascontiguousarray(states[c])}
        for c in range(N_CORES)
    ]
    res = run_bass_kernel_spmd(nc, in_maps, core_ids=list(range(N_CORES)))
    out = np.concatenate([res.results[c]["out"] for c in range(N_CORES)], axis=0)
    assert out.shape == (N_ROWS, COUNT) and out.dtype == np.uint8, (out.shape, out.dtype)
    return out


if __name__ == "__main__":
    rng = np.random.default_rng(0)
    probs = rng.random((1, N_ROWS), dtype=np.float32)
    out = kernel(probs, COUNT)
    exp = model_output(probs.reshape(-1))
    print("exact model match:", np.array_equal(out, exp))
    print("mean bits:", np.unpackbits(out, axis=1).mean(), "target:", probs.mean())


# revision 23
# speedup vs baseline: 6.1961x; 6.1961x over previous
"""Bitpacked Bernoulli sampling kernel for Trainium2 (8 NeuronCores, SPMD).

Problem: probs [1, 32768] f32, count=256 -> output [32768, 256] uint8, where
byte (n, c) packs 8 Bernoulli(probs[n]) bits MSB-first.

Strategy (per the sharding hint): embarrassingly parallel over the probs axis.
Each of the 8 cores handles 4096 rows and draws its own RNG stream from the
DVE's hardware xorwow generator (register-seeded per core, deterministic).

Per tile of 512 rows (128 partitions x 4 row-segments x 2048 bits):
  - DVE `random` fills r [128, 8192] u32 (~0.1 us)
  - ScalarE Sign-activation compares each segment against its per-partition
    threshold: bits_u8 = u8(Sign(-r + p*2^32)); negatives saturate to 0,
    Sign(0)=0, so this is exactly (f32(r) < thr).
  - Bits are laid out bit-plane-major (plane b holds the weight-2^(7-b) bit
    of all 256 bytes), so packing is 3 shift/or levels on *uint32 views*,
    touching 4 bytes per element: values stay < 128 so shifts by 1/2/4 never
    cross byte lanes.
  - DMA out: rows t*512 + p*4 + s are contiguous 1024-byte blocks per p.
"""
import sys

for _p in ("/opt/trn_rl_repo", "/root/.axon_site/_ro/trn_rl_repo"):
    if _p not in sys.path:
        sys.path.insert(0, _p)

import numpy as np

N_ROWS = 32768
COUNT = 256
N_CORES = 8
ROWS_PER_CORE = N_ROWS // N_CORES          # 4096
N_TILES = 8                                # tiles per core
ROWS_PER_TILE = ROWS_PER_CORE // N_TILES   # 512 rows: 128 partitions x 4 segments
SEGS = 4                                   # rows per partition per tile
BITS_PER_ROW = COUNT * 8                   # 2048
TILE_F = SEGS * BITS_PER_ROW               # 8192 uniforms per partition per tile
SEED = 1234

_CACHE = {}


def _build_program(debug_rin=False):
    import concourse.bass as bass
    import concourse.mybir as mybir
    from contextlib import ExitStack

    Alu = mybir.AluOpType
    u32 = mybir.dt.uint32
    u8 = mybir.dt.uint8
    f32 = mybir.dt.float32
    Act = mybir.ActivationFunctionType

    def stt_imm(eng, out, in0, v, in1, op0, op1, imm_dtype):
        return eng.add_instruction(
            mybir.InstTensorScalarPtr(
                name=eng.bass.get_next_instruction_name(),
                is_scalar_tensor_tensor=True,
                op0=op0, op1=op1,
                ins=[eng.lower_ap(in0),
                     mybir.ImmediateValue(dtype=imm_dtype, value=v),
                     eng.lower_ap(in1)],
                outs=[eng.lower_ap(out)],
            )
        )

    nc = bass.Bass("TRN2", target_bir_lowering=False, debug=False)
    thr_d = nc.dram_tensor("thr", [128, N_TILES * SEGS], f32, kind="ExternalInput")
    seed_d = nc.dram_tensor("seed", [1, 1], u32, kind="ExternalInput")
    out_d = nc.dram_tensor("out", [ROWS_PER_CORE, COUNT], u8, kind="ExternalOutput")
    if debug_rin:
        rin_d = nc.dram_tensor("rin", [128, N_TILES * TILE_F], u32,
                               kind="ExternalInput")

    NB = 2
    RNG_INC = 16 if debug_rin else 1
    W32 = TILE_F // 4          # 2048 u32 words per partition in the bits tile
    SEG32 = W32 // SEGS        # 512 u32 words per segment
    PL32 = SEG32 // 8          # 64 u32 words per bit-plane

    with ExitStack() as ctx:
        thr_t = ctx.enter_context(nc.sbuf_tensor("thr_t", [128, N_TILES * SEGS], f32))
        seed_t = ctx.enter_context(nc.sbuf_tensor("seed_t", [1, 1], u32))
        r_b = [ctx.enter_context(nc.sbuf_tensor(f"r{i}", [128, TILE_F], u32))
               for i in range(NB)]
        # bits native u8 (ACT 2x mode needs a native-dtype output AP);
        # pack reads view them as u32 (4 packed bytes per element)
        bits_b = [ctx.enter_context(nc.sbuf_tensor(f"bits{i}", [128, TILE_F], u8))
                  for i in range(NB)]
        t1_t = ctx.enter_context(nc.sbuf_tensor("t1", [128, W32 // 2], u32))
        t2_t = ctx.enter_context(nc.sbuf_tensor("t2", [128, W32 // 4], u32))
        pk_b = [ctx.enter_context(nc.sbuf_tensor(f"pk{i}", [128, W32 // 8], u32))
                for i in range(NB)]
        rgate_t = ctx.enter_context(nc.sbuf_tensor("rgate", [128, 8], u32))
        s_in = ctx.enter_context(nc.semaphore("s_in"))
        s_sd = ctx.enter_context(nc.semaphore("s_sd"))
        s_rng = ctx.enter_context(nc.semaphore("s_rng"))
        s_act = ctx.enter_context(nc.semaphore("s_act"))
        s_l1 = ctx.enter_context(nc.semaphore("s_l1"))
        s_tile = ctx.enter_context(nc.semaphore("s_tile"))
        s_out = ctx.enter_context(nc.semaphore("s_out"))
        block = ctx.enter_context(nc.Block())

        @block.sync
        def _(sync):
            sync.dma_start(seed_t[:, :], seed_d[:, :]).then_inc(s_sd, 16)
            sync.dma_start(thr_t[:, :], thr_d[:, :]).then_inc(s_in, 16)
            def emit_out(t):
                sync.wait_ge(s_tile, t + 1)
                dst = bass.AP(out_d, t * ROWS_PER_TILE * COUNT,
                              [[SEGS * COUNT, 128], [1, SEGS * COUNT]])
                src = bass.AP(pk_b[t % NB], 0,
                              [[W32 // 8, 128], [1, W32 // 8]]).bitcast(u8)
                sync.dma_start(dst, src).then_inc(s_out, 16)

            if debug_rin:
                # interleave rin loads with out stores so the pack's s_out
                # waits can make progress (same sync queue)
                for t in range(N_TILES):
                    sync.wait_ge(s_act, 4 * max(0, t - NB + 1))
                    sync.dma_start(
                        r_b[t % NB][:, :],
                        bass.AP(rin_d, t * TILE_F, [[N_TILES * TILE_F, 128],
                                                    [1, TILE_F]]),
                    ).then_inc(s_rng, 16)
                    if t >= NB:
                        emit_out(t - NB)
                for t in range(N_TILES - NB, N_TILES):
                    emit_out(t)
            else:
                for t in range(N_TILES):
                    emit_out(t)

        @block.vector
        def _(vector):
            def emit_rng(t):
                # Big fill, then a tiny 8-word fill as a completion gate: the
                # second RNG cannot start until the first fill has drained, so
                # its retirement (and the sem inc) proves r[t] is fully
                # written before ScalarE starts reading it.
                vector.random(r_b[t % NB][:, :])
                vector.random(rgate_t[:, :]).then_inc(s_rng, 1)

            if not debug_rin:
                vector.wait_ge(s_sd, 16)
                sv = vector.value_load(seed_t[0:1, 0:1])
                ra = vector.lower_val_access(sv)
                vector.add_instruction(
                    mybir.InstSetRandState(
                        name=nc.get_next_instruction_name(),
                        ins=[ra],
                        outs=[vector._lower_rng_state_ap()],
                        rng_engine=vector.engine.value,
                    )
                )
                emit_rng(0)
                emit_rng(1)
            for t in range(N_TILES):
                # ACT(t) done: r[t % NB] is free and tile t is ready to pack
                vector.wait_ge(s_act, 4 * (t + 1))
                if not debug_rin and t + 2 < N_TILES:
                    # trigger tile t+2's fill FIRST so it runs in the
                    # background while we pack tile t
                    vector.random(r_b[(t + 2) % NB][:, :])
                bits = bits_b[t % NB]
                # L1: planes (0,2,4,6) vs (1,3,5,7):  t1 = (even << 1) | odd
                in0 = bass.AP(bits, 0, [[TILE_F, 128], [4 * SEG32, SEGS],
                                        [8 * PL32, 4], [1, 4 * PL32]]).bitcast(u32)
                in1 = bass.AP(bits, 4 * PL32, [[TILE_F, 128], [4 * SEG32, SEGS],
                                               [8 * PL32, 4], [1, 4 * PL32]]).bitcast(u32)
                o1 = bass.AP(t1_t, 0, [[W32 // 2, 128], [1, W32 // 2]])
                stt_imm(vector, o1, in0, 1, in1,
                        Alu.logical_shift_left, Alu.bitwise_or,
                        u32).then_inc(s_l1, 1)
                # L2: t2 = (t1 pairs even << 2) | odd ; t1 layout [s][m(4)][64]
                i20 = bass.AP(t1_t, 0, [[W32 // 2, 128], [SEG32 // 2, SEGS], [2 * PL32, 2], [1, PL32]])
                i21 = bass.AP(t1_t, PL32, [[W32 // 2, 128], [SEG32 // 2, SEGS], [2 * PL32, 2], [1, PL32]])
                o2 = bass.AP(t2_t, 0, [[W32 // 4, 128], [1, W32 // 4]])
                stt_imm(vector, o2, i20, 2, i21,
                        Alu.logical_shift_left, Alu.bitwise_or, u32)
                # L3: pk = (t2 pairs even << 4) | odd ; t2 layout [s][m(2)][64]
                i30 = bass.AP(t2_t, 0, [[W32 // 4, 128], [SEG32 // 4, SEGS], [2 * PL32, 1], [1, PL32]])
                i31 = bass.AP(t2_t, PL32, [[W32 // 4, 128], [SEG32 // 4, SEGS], [2 * PL32, 1], [1, PL32]])
                o3 = bass.AP(pk_b[t % NB], 0, [[W32 // 8, 128], [1, W32 // 8]])
                if t >= NB:
                    vector.wait_ge(s_out, 16 * (t - NB + 1))
                stt_imm(vector, o3, i30, 4, i31,
                        Alu.logical_shift_left, Alu.bitwise_or,
                        u32).then_inc(s_tile, 1)
                if not debug_rin and t + 2 < N_TILES:
                    # completion gate for tile t+2's fill
                    vector.random(rgate_t[:, :]).then_inc(s_rng, 1)

        @block.scalar
        def _(scalar):
            scalar.wait_ge(s_in, 16)
            for t in range(N_TILES):
                scalar.wait_ge(s_rng, RNG_INC * (t + 1))
                if t >= NB:
                    # bits[t%NB] free once L1 of tile t-NB has read it
                    scalar.wait_ge(s_l1, t - NB + 1)
                r = r_b[t % NB]
                bits = bits_b[t % NB]
                for s in range(SEGS):
                    # bits segment s as u8 view of the u32 tile
                    scalar.activation(
                        bits[:, s * BITS_PER_ROW:(s + 1) * BITS_PER_ROW],
                        r[:, s * BITS_PER_ROW:(s + 1) * BITS_PER_ROW],
                        Act.Sign,
                        bias=thr_t[:, t * SEGS + s: t * SEGS + s + 1],
                        scale=-1.0,
                    ).then_inc(s_act, 1)

    return nc


def make_seeds(seed=SEED):
    ss = np.random.SeedSequence(seed)
    gen = np.random.Generator(np.random.PCG64(ss))
    return gen.integers(1, 2**32 - 1, size=(N_CORES,), dtype=np.uint32)


def make_thresholds(probs):
    """Thresholds p * 2^32 (f32) laid out [core][128, 32]: row = c*4096 +
    t*512 + p*4 + s -> thr[core][p, t*4+s]."""
    v = (probs.reshape(N_CORES, N_TILES, 128, SEGS).astype(np.float64)
         * 4294967296.0).astype(np.float32)
    return np.ascontiguousarray(v.transpose(0, 2, 1, 3).reshape(N_CORES, 128, N_TILES * SEGS))


def postprocess_model(r_core, thr_core):
    """Numpy model of compare+pack+layout for ONE core given its raw random
    stream r_core [128, N_TILES*TILE_F] u32. Returns [4096, 256] u8."""
    out = np.empty((ROWS_PER_CORE, COUNT), np.uint8)
    for t in range(N_TILES):
        r = r_core[:, t * TILE_F:(t + 1) * TILE_F]
        for s in range(SEGS):
            seg = r[:, s * BITS_PER_ROW:(s + 1) * BITS_PER_ROW]  # [128, 2048]
            thr = thr_core[:, t * SEGS + s][:, None]
            bits = (seg.astype(np.float32) < thr).astype(np.uint8)
            planes = bits.reshape(128, 8, COUNT)                 # [p, b, c]
            w = (np.uint8(128) >> np.arange(8, dtype=np.uint8))[None, :, None]
            byts = (planes * w).sum(axis=1).astype(np.uint8)     # [p, c]
            rows = t * ROWS_PER_TILE + np.arange(128) * SEGS + s
            out[rows] = byts
    return out


def _get_program():
    if "nc" not in _CACHE:
        _CACHE["nc"] = _build_program()
    return _CACHE["nc"]


def kernel(probs_batch, count):
    probs_batch = np.asarray(probs_batch)
    assert probs_batch.shape == (1, N_ROWS), probs_batch.shape
    assert int(count) == COUNT, count
    probs = probs_batch.reshape(N_ROWS).astype(np.float32)

    from concourse.bass_utils import run_bass_kernel_spmd

    nc = _get_program()
    thr = make_thresholds(probs)
    seeds = make_seeds()
    in_maps = [
        {"thr": np.ascontiguousarray(thr[c]),
         "seed": np.array([[seeds[c]]], dtype=np.uint32)}
        for c in range(N_CORES)
    ]
    res = run_bass_kernel_spmd(nc, in_maps, core_ids=list(range(N_CORES)))
    out = np.concatenate([res.results[c]["out"] for c in range(N_CORES)], axis=0)
    assert out.shape == (N_ROWS, COUNT) and out.dtype == np.uint8, (out.shape, out.dtype)
    return out


if __name__ == "__main__":
    rng = np.random.default_rng(0)
    probs = rng.random((1, N_ROWS), dtype=np.float32)
    out = kernel(probs, COUNT)
    out2 = kernel(probs, COUNT)
    print("deterministic:", np.array_equal(out, out2))
    print("mean bits:", np.unpackbits(out, axis=1).mean(), "target:", probs.mean())
    # cross-row duplicate check (RNG stream independence)
    v = out.reshape(N_ROWS, -1)
    uu = np.unique(v, axis=0)
    print("unique rows:", uu.shape[0], "/", N_ROWS)
